# revision 19
# baseline (speedup 1.0000x reference)
import sys

if "/opt/trn_rl_repo" not in sys.path:
    sys.path.insert(0, "/opt/trn_rl_repo")

import numpy as np

import concourse.bass as bass
import concourse.mybir as mybir
import concourse.tile as tile
from concourse import bacc
from concourse.bass_utils import run_bass_kernel_spmd
from concourse.masks import make_identity

P = 128
S = 2048          # sequence length (keys) per batch
D = 1024          # d_model
H = 16            # heads
DH = 64           # d_head
Q = 512           # query rows per core
N_CORES = 8
F32 = mybir.dt.float32
BF16 = mybir.dt.bfloat16
EXPF = mybir.ActivationFunctionType.Exp
SQRTF = mybir.ActivationFunctionType.Sqrt
MULT = mybir.AluOpType.mult
ADD = mybir.AluOpType.add
SUB = mybir.AluOpType.subtract


def build():
    nc = bacc.Bacc("TRN2", target_bir_lowering=False, debug=False,
                   num_devices=N_CORES)

    x_in = nc.dram_tensor("x", [S, D], F32, kind="ExternalInput")
    wq_in = nc.dram_tensor("Wq", [D, D], F32, kind="ExternalInput")
    wk_in = nc.dram_tensor("Wk", [D, D], F32, kind="ExternalInput")
    wv_in = nc.dram_tensor("Wv", [D, DH], F32, kind="ExternalInput")
    wo_in = nc.dram_tensor("Wo", [D, D], F32, kind="ExternalInput")
    g_in = nc.dram_tensor("gamma", [D], F32, kind="ExternalInput")
    b_in = nc.dram_tensor("beta", [D], F32, kind="ExternalInput")
    out_o = nc.dram_tensor("out", [Q, D], F32, kind="ExternalOutput")
    avg_o = nc.dram_tensor("avg", [Q, S], F32, kind="ExternalOutput")

    with tile.TileContext(nc) as tc:
        with (
            tc.tile_pool(name="persist", bufs=1) as pp,
            tc.tile_pool(name="dram", bufs=1, space="DRAM") as dp,
        ):
            allones = pp.tile([P, P], BF16)
            nc.gpsimd.memset(allones[:], 1.0)

            # persistent activations
            kT = pp.tile([P, 8, S], BF16)      # K^T: [dh-chunk(2 heads), 8, S]
            qT = pp.tile([P, 8, Q], BF16)      # Q^T
            vaug_e = pp.tile([P, 16, 65], BF16)  # [V | 1]
            ctxT = pp.tile([P, 8, Q], BF16)    # normalized context^T
            acc = pp.tile([P, 16, Q], BF16)     # sum_h exp/r  (k-part, kc, q)
            wo_dram = dp.tile([P, 8, D], BF16)
            x_bf_dram = dp.tile([S, D], BF16)

            nc.gpsimd.memset(vaug_e[:], 1.0)

            # ---------------- Phase A+B: load/convert/transpose + projections
            with (
                tc.tile_pool(name="proj", bufs=1) as jp,
                tc.tile_pool(name="work", bufs=3) as wp,
                tc.tile_pool(name="ps1", bufs=1, space="PSUM") as ps1,
            ):
                xT = jp.tile([P, 8, S], BF16)   # x^T (freed after phase B)
                wq_bf = jp.tile([P, 8, D], BF16)
                wk_bf = jp.tile([P, 8, D], BF16)
                wv_bf = jp.tile([P, 8, DH], BF16)

                # weights: load + bf16 convert
                for dc in range(8):
                    for src, dst in ((wq_in, wq_bf), (wk_in, wk_bf)):
                        wf = wp.tile([P, D], F32, tag="wf")
                        nc.sync.dma_start(wf[:], src.ap()[dc * P:(dc + 1) * P, :])
                        nc.any.tensor_copy(dst[:, dc, :], wf[:])
                    wvf = wp.tile([P, DH], F32, tag="wvf")
                    nc.sync.dma_start(wvf[:], wv_in.ap()[dc * P:(dc + 1) * P, :])
                    nc.any.tensor_copy(wv_bf[:, dc, :], wvf[:])
                    # Wo -> bf16 -> DRAM scratch (reloaded in final phase)
                    wof = wp.tile([P, D], F32, tag="wf")
                    nc.sync.dma_start(wof[:], wo_in.ap()[dc * P:(dc + 1) * P, :])
                    wob = wp.tile([P, D], BF16, tag="wob")
                    nc.any.tensor_copy(wob[:], wof[:])
                    nc.sync.dma_start(wo_dram[:, dc, :], wob[:])

                # x: load rows, convert to bf16, bounce through DRAM, then
                # DMA-xbar transpose into xT (keeps TensorE free)
                for sc in range(16):
                    xf = wp.tile([P, D], F32, tag="xf")
                    nc.sync.dma_start(xf[:], x_in.ap()[sc * P:(sc + 1) * P, :])
                    xb = wp.tile([P, D], BF16, tag="xb")
                    nc.any.tensor_copy(xb[:], xf[:])
                    nc.sync.dma_start(x_bf_dram[sc * P:(sc + 1) * P, :], xb[:])
                for dc in range(8):
                    nc.sync.dma_start_transpose(xT[:, dc, :],
                                                x_bf_dram[:, dc * P:(dc + 1) * P])

                # K^T projection: kT[:,oc,s] = sum_dc Wk[dc,oc-chunk]^T x^T
                for oc in range(8):
                    for sb in range(4):
                        ps = ps1.tile([P, 512], F32, tag="kq", bufs=3)
                        for dc in range(8):
                            nc.tensor.matmul(
                                ps[:], wk_bf[:, dc, oc * P:(oc + 1) * P],
                                xT[:, dc, sb * 512:(sb + 1) * 512],
                                start=(dc == 0), stop=(dc == 7))
                        nc.any.tensor_copy(kT[:, oc, sb * 512:(sb + 1) * 512],
                                           ps[:])
                # Q^T projection (queries are rows 0:Q)
                for oc in range(8):
                    ps = ps1.tile([P, 512], F32, tag="kq", bufs=3)
                    for dc in range(8):
                        nc.tensor.matmul(ps[:], wq_bf[:, dc, oc * P:(oc + 1) * P],
                                         xT[:, dc, 0:Q],
                                         start=(dc == 0), stop=(dc == 7))
                    nc.any.tensor_copy(qT[:, oc, :], ps[:])
                # V projection: V[k,dh] natural layout
                for kc in range(16):
                    psv = ps1.tile([P, DH], F32, tag="v", bufs=2)
                    for dc in range(8):
                        nc.tensor.matmul(psv[:], xT[:, dc, kc * P:(kc + 1) * P],
                                         wv_bf[:, dc, :],
                                         start=(dc == 0), stop=(dc == 7))
                    nc.any.tensor_copy(vaug_e[:, kc, 0:DH], psv[:])

            # ---------------- Phase C: attention (4 batches of 4 heads)
            with (
                tc.tile_pool(name="attn", bufs=1) as ap_,
                tc.tile_pool(name="pss", bufs=1, space="PSUM") as pss,
            ):
                first = True
                for bat in range(4):
                    batch_sums = []
                    for pi in range(2):
                        pr = 2 * bat + pi
                        exp_e = ap_.tile([P, 16, Q], BF16, tag="exp", bufs=5,
                                         name=f"exp_e{pr}")
                        exp_o = ap_.tile([P, 16, Q], BF16, tag="exp", bufs=5,
                                         name=f"exp_o{pr}")
                        ps_ce = pss.tile([P, 512], F32, tag="c", bufs=4,
                                         name=f"ctx_e{pr}")
                        ps_co = pss.tile([P, 512], F32, tag="c", bufs=4,
                                         name=f"ctx_o{pr}")
                        for kc2 in range(8):
                            ps_e = pss.tile([P, 2, 512], F32, tag="s", bufs=2,
                                            name="score_e")
                            ps_o = pss.tile([P, 2, 512], F32, tag="s", bufs=2,
                                            name="score_o")
                            for hf in range(2):
                                kc = 2 * kc2 + hf
                                nc.tensor.matmul(
                                    ps_e[:, hf, :],
                                    kT[0:DH, pr, kc * P:(kc + 1) * P],
                                    qT[0:DH, pr, :], start=True, stop=True)
                                nc.tensor.matmul(
                                    ps_o[:, hf, :],
                                    kT[DH:P, pr, kc * P:(kc + 1) * P],
                                    qT[DH:P, pr, :], start=True, stop=True)
                            nc.scalar.activation(
                                exp_e[:, 2 * kc2:2 * kc2 + 2, :], ps_e[:],
                                EXPF, scale=0.125)
                            nc.scalar.activation(
                                exp_o[:, 2 * kc2:2 * kc2 + 2, :], ps_o[:],
                                EXPF, scale=0.125)
                            for hf in range(2):
                                kc = 2 * kc2 + hf
                                nc.tensor.matmul(
                                    ps_ce[0:65, :], vaug_e[:, kc, :],
                                    exp_e[:, kc, :],
                                    start=(kc == 0), stop=(kc == 15))
                                nc.tensor.matmul(
                                    ps_co[0:65, :], vaug_e[:, kc, :],
                                    exp_o[:, kc, :],
                                    start=(kc == 0), stop=(kc == 15))
                        # per-pair tail: rowsums -> 1/r -> replicate -> scale
                        rs2 = ap_.tile([P, 2, 512], F32, tag="rs64", bufs=2)
                        nc.any.tensor_copy(rs2[64:65, 0, :], ps_ce[64:65, :])
                        nc.any.tensor_copy(rs2[64:65, 1, :], ps_co[64:65, :])
                        rsp = ap_.tile([2, 512], F32, tag="rs", bufs=2)
                        nc.sync.dma_start(rsp[:], rs2[64:65, :, :])
                        inv2 = ap_.tile([2, 512], F32, tag="inv", bufs=2)
                        nc.vector.reciprocal(inv2[:], rsp[:])
                        inv2b = ap_.tile([2, 512], BF16, tag="invb", bufs=2)
                        nc.any.tensor_copy(inv2b[:], inv2[:])
                        invb0 = ap_.tile([1, 2, 512], BF16, tag="invb0", bufs=2)
                        nc.sync.dma_start(invb0[:], inv2b[:])
                        for j, (ex, pc) in enumerate(
                                ((exp_e, ps_ce), (exp_o, ps_co))):
                            ps_r = pss.tile([P, 512], F32, tag="s", bufs=2,
                                            name="invrep")
                            nc.tensor.matmul(ps_r[:], allones[0:1, :],
                                             invb0[0:1, j, :],
                                             start=True, stop=True)
                            irb = ap_.tile([P, 512], BF16, tag="irb", bufs=2)
                            nc.any.tensor_copy(irb[:], ps_r[:])
                            if j == 0:
                                nc.vector.tensor_tensor(
                                    ctxT[0:DH, pr, :], pc[0:DH, :],
                                    irb[0:DH, :], op=MULT)
                            else:
                                tmpc = ap_.tile([DH, 512], BF16, tag="tmpc",
                                                bufs=2)
                                nc.vector.tensor_tensor(
                                    tmpc[:], pc[0:DH, :],
                                    irb[0:DH, :], op=MULT)
                                nc.sync.dma_start(ctxT[DH:P, pr, :], tmpc[:])
                            nc.vector.tensor_tensor(
                                ex[:], ex[:],
                                irb[:, None, :].to_broadcast([P, 16, Q]),
                                op=MULT)
                        nc.vector.tensor_tensor(exp_e[:], exp_e[:], exp_o[:],
                                                op=ADD)
                        batch_sums.append(exp_e)
                    e0, e2 = batch_sums
                    nc.vector.tensor_tensor(e0[:], e0[:], e2[:], op=ADD)
                    if first:
                        nc.any.tensor_copy(acc[:], e0[:])
                        first = False
                    else:
                        nc.vector.tensor_tensor(acc[:], acc[:], e0[:], op=ADD)

            # ---------------- Phase D: out-proj + residual + LayerNorm
            with (
                tc.tile_pool(name="fin", bufs=1) as fp,
                tc.tile_pool(name="wrk2", bufs=2) as wp2,
                tc.tile_pool(name="psf", bufs=1, space="PSUM") as psf,
            ):
                wo2 = fp.tile([P, 8, D], BF16)
                nc.sync.dma_start(wo2[:], wo_dram[:])
                g_rep = fp.tile([P, D], F32)
                b_rep = fp.tile([P, D], F32)
                nc.sync.dma_start(
                    g_rep[:],
                    g_in.ap().rearrange("(a d) -> a d", a=1).to_broadcast([P, D]))
                nc.sync.dma_start(
                    b_rep[:],
                    b_in.ap().rearrange("(a d) -> a d", a=1).to_broadcast([P, D]))
                eps_t = fp.tile([P, 1], F32)
                nc.gpsimd.memset(eps_t[:], 1e-6)
                x4 = fp.tile([P, 4, D], F32)
                for qc in range(4):
                    nc.sync.dma_start(x4[:, qc, :],
                                      x_in.ap()[qc * P:(qc + 1) * P, :])
                for qc in range(4):
                    pso = psf.tile([P, D], F32, tag="o", bufs=2)
                    for nh in range(2):
                        for dc in range(8):
                            nc.tensor.matmul(
                                pso[:, nh * 512:(nh + 1) * 512],
                                ctxT[:, dc, qc * P:(qc + 1) * P],
                                wo2[:, dc, nh * 512:(nh + 1) * 512],
                                start=(dc == 0), stop=(dc == 7))
                    y = wp2.tile([P, D], F32, tag="y")
                    ysum = wp2.tile([P, 1], F32, tag="ys")
                    nc.vector.scalar_tensor_tensor(
                        out=y[:], in0=pso[:], scalar=1.0, in1=x4[:, qc, :],
                        op0=MULT, op1=ADD, accum_out=ysum[:])
                    mu = wp2.tile([P, 1], F32, tag="mu")
                    nc.vector.tensor_scalar_mul(mu[:], ysum[:], 1.0 / D)
                    scr = wp2.tile([P, D], BF16, tag="scr")
                    vsum = wp2.tile([P, 1], F32, tag="vs")
                    nc.vector.scalar_tensor_tensor(
                        out=scr[:], in0=y[:], scalar=mu[:], in1=y[:],
                        op0=SUB, op1=MULT, accum_out=vsum[:])
                    std = wp2.tile([P, 1], F32, tag="sd")
                    nc.scalar.activation(std[:], vsum[:], SQRTF,
                                         bias=eps_t[:], scale=1.0 / D)
                    istd = wp2.tile([P, 1], F32, tag="is")
                    nc.vector.reciprocal(istd[:], std[:])
                    nrm = wp2.tile([P, D], F32, tag="nrm")
                    nc.vector.scalar_tensor_tensor(
                        out=nrm[:], in0=y[:], scalar=mu[:],
                        in1=istd[:].to_broadcast([P, D]), op0=SUB, op1=MULT)
                    nc.vector.tensor_tensor(nrm[:], nrm[:], g_rep[:], op=MULT)
                    nc.vector.tensor_tensor(nrm[:], nrm[:], b_rep[:], op=ADD)
                    nc.sync.dma_start(out_o.ap()[qc * P:(qc + 1) * P, :], nrm[:])

                # avg_weights: DMA-xbar transpose bf16 acc -> [q, k],
                # then one convert+scale (1/H) pass to fp32 per q-block
                for qb in range(4):
                    stgb = fp.tile([P, S], BF16, tag="stgb", bufs=2)
                    for kc in range(16):
                        nc.sync.dma_start_transpose(
                            stgb[:, kc * P:(kc + 1) * P],
                            acc[:, kc, qb * P:(qb + 1) * P])
                    stg = fp.tile([P, S], F32, tag="stg", bufs=2)
                    nc.scalar.mul(stg[:], stgb[:], 1.0 / H)
                    nc.sync.dma_start(avg_o.ap()[qb * P:(qb + 1) * P, :], stg[:])

    nc.compile()
    return nc


_NC = None


def _get_nc():
    global _NC
    if _NC is None:
        _NC = build()
    return _NC


def _make_in_maps(inputs):
    x = np.ascontiguousarray(np.asarray(inputs["x"], dtype=np.float32))
    Wq = np.ascontiguousarray(np.asarray(inputs["Wq"], dtype=np.float32))
    Wk = np.ascontiguousarray(np.asarray(inputs["Wk"], dtype=np.float32))
    Wv = np.ascontiguousarray(np.asarray(inputs["Wv"], dtype=np.float32))
    Wo = np.ascontiguousarray(np.asarray(inputs["Wo"], dtype=np.float32))
    g = np.ascontiguousarray(np.asarray(inputs["ln_gamma"], dtype=np.float32))
    bt = np.ascontiguousarray(np.asarray(inputs["ln_beta"], dtype=np.float32))
    in_maps, perms = [], []
    for c in range(N_CORES):
        b_idx, j = divmod(c, 4)
        q0 = j * Q
        perm = np.concatenate([np.arange(q0, q0 + Q), np.arange(0, q0),
                               np.arange(q0 + Q, S)])
        xp = np.ascontiguousarray(x[b_idx][perm])
        in_maps.append({"x": xp, "Wq": Wq, "Wk": Wk, "Wv": Wv, "Wo": Wo,
                        "gamma": g, "beta": bt})
        perms.append(perm)
    return in_maps, perms


def run(inputs, trace=False):
    nc = _get_nc()
    in_maps, perms = _make_in_maps(inputs)
    res = run_bass_kernel_spmd(nc, in_maps, list(range(N_CORES)),
                               trace=trace, trace_cores=[0] if trace else None)
    out = np.empty((2, S, D), np.float32)
    avg = np.empty((2, S, S), np.float32)
    for c in range(N_CORES):
        b_idx, j = divmod(c, 4)
        q0 = j * Q
        out[b_idx, q0:q0 + Q] = res.results[c]["out"]
        avg[b_idx, q0:q0 + Q][:, perms[c]] = res.results[c]["avg"]
    return (out, avg), res.exec_time_ns


def kernel(**inputs):
    (out, avg), _ = run(inputs, trace=False)
    return out, avg


# revision 20
# speedup vs baseline: 1.2363x; 1.2363x over previous
import sys

if "/opt/trn_rl_repo" not in sys.path:
    sys.path.insert(0, "/opt/trn_rl_repo")

import numpy as np

import concourse.bass as bass
import concourse.mybir as mybir
import concourse.tile as tile
from concourse import bacc
from concourse.bass_utils import run_bass_kernel_spmd
from concourse.masks import make_identity

P = 128
S = 2048          # sequence length (keys) per batch
D = 1024          # d_model
H = 16            # heads
DH = 64           # d_head
Q = 512           # query rows per core
N_CORES = 8
F32 = mybir.dt.float32
BF16 = mybir.dt.bfloat16
EXPF = mybir.ActivationFunctionType.Exp
SQRTF = mybir.ActivationFunctionType.Sqrt
MULT = mybir.AluOpType.mult
ADD = mybir.AluOpType.add
SUB = mybir.AluOpType.subtract


def build():
    nc = bacc.Bacc("TRN2", target_bir_lowering=False, debug=False,
                   num_devices=N_CORES)

    x_in = nc.dram_tensor("x", [S, D], F32, kind="ExternalInput")
    wq_in = nc.dram_tensor("Wq", [D, D], F32, kind="ExternalInput")
    wk_in = nc.dram_tensor("Wk", [D, D], F32, kind="ExternalInput")
    wv_in = nc.dram_tensor("Wv", [D, DH], F32, kind="ExternalInput")
    wo_in = nc.dram_tensor("Wo", [D, D], F32, kind="ExternalInput")
    g_in = nc.dram_tensor("gamma", [D], F32, kind="ExternalInput")
    b_in = nc.dram_tensor("beta", [D], F32, kind="ExternalInput")
    out_o = nc.dram_tensor("out", [Q, D], F32, kind="ExternalOutput")
    avg_o = nc.dram_tensor("avg", [Q, S], F32, kind="ExternalOutput")

    with tile.TileContext(nc) as tc:
        with (
            tc.tile_pool(name="persist", bufs=1) as pp,
            tc.tile_pool(name="dram", bufs=1, space="DRAM") as dp,
        ):
            ident_b = pp.tile([P, P], BF16)
            allones = pp.tile([P, P], BF16)
            make_identity(nc, ident_b)
            nc.gpsimd.memset(allones[:], 1.0)

            # persistent activations
            kT = pp.tile([P, 8, S], BF16)      # K^T: [dh-chunk(2 heads), 8, S]
            qT = pp.tile([P, 8, Q], BF16)      # Q^T
            vaug_e = pp.tile([P, 16, 65], BF16)  # [V | 1]
            ctxT = pp.tile([P, 8, Q], BF16)    # normalized context^T
            acc = pp.tile([P, 16, Q], BF16)     # sum_h exp/r  (k-part, kc, q)
            wo_dram = dp.tile([P, 8, D], BF16)
            x_bf_dram = dp.tile([S, D], BF16)

            nc.gpsimd.memset(vaug_e[:], 1.0)

            # ---------------- Phase A+B: load/convert/transpose + projections
            with (
                tc.tile_pool(name="proj", bufs=1) as jp,
                tc.tile_pool(name="work", bufs=3) as wp,
                tc.tile_pool(name="ps1", bufs=1, space="PSUM") as ps1,
            ):
                xT = jp.tile([P, 8, S], BF16)   # x^T (freed after phase B)
                wq_bf = jp.tile([P, 8, D], BF16)
                wk_bf = jp.tile([P, 8, D], BF16)
                wv_bf = jp.tile([P, 8, DH], BF16)

                # weights: load + bf16 convert
                for dc in range(8):
                    for src, dst in ((wq_in, wq_bf), (wk_in, wk_bf)):
                        wf = wp.tile([P, D], F32, tag="wf")
                        nc.sync.dma_start(wf[:], src.ap()[dc * P:(dc + 1) * P, :])
                        nc.any.tensor_copy(dst[:, dc, :], wf[:])
                    wvf = wp.tile([P, DH], F32, tag="wvf")
                    nc.sync.dma_start(wvf[:], wv_in.ap()[dc * P:(dc + 1) * P, :])
                    nc.any.tensor_copy(wv_bf[:, dc, :], wvf[:])
                    # Wo -> bf16 -> DRAM scratch (reloaded in final phase)
                    wof = wp.tile([P, D], F32, tag="wf")
                    nc.sync.dma_start(wof[:], wo_in.ap()[dc * P:(dc + 1) * P, :])
                    wob = wp.tile([P, D], BF16, tag="wob")
                    nc.any.tensor_copy(wob[:], wof[:])
                    nc.sync.dma_start(wo_dram[:, dc, :], wob[:])

                # x: load rows, convert to bf16, bounce through DRAM, then
                # DMA-xbar transpose into xT (keeps TensorE free)
                for sc in range(16):
                    xf = wp.tile([P, D], F32, tag="xf")
                    nc.sync.dma_start(xf[:], x_in.ap()[sc * P:(sc + 1) * P, :])
                    xb = wp.tile([P, D], BF16, tag="xb")
                    nc.any.tensor_copy(xb[:], xf[:])
                    nc.sync.dma_start(x_bf_dram[sc * P:(sc + 1) * P, :], xb[:])
                for dc in range(8):
                    nc.sync.dma_start_transpose(xT[:, dc, :],
                                                x_bf_dram[:, dc * P:(dc + 1) * P])

                # K^T projection: kT[:,oc,s] = sum_dc Wk[dc,oc-chunk]^T x^T
                for oc in range(8):
                    for sb in range(4):
                        ps = ps1.tile([P, 512], F32, tag="kq", bufs=3)
                        for dc in range(8):
                            nc.tensor.matmul(
                                ps[:], wk_bf[:, dc, oc * P:(oc + 1) * P],
                                xT[:, dc, sb * 512:(sb + 1) * 512],
                                start=(dc == 0), stop=(dc == 7))
                        nc.any.tensor_copy(kT[:, oc, sb * 512:(sb + 1) * 512],
                                           ps[:])
                # Q^T projection (queries are rows 0:Q)
                for oc in range(8):
                    ps = ps1.tile([P, 512], F32, tag="kq", bufs=3)
                    for dc in range(8):
                        nc.tensor.matmul(ps[:], wq_bf[:, dc, oc * P:(oc + 1) * P],
                                         xT[:, dc, 0:Q],
                                         start=(dc == 0), stop=(dc == 7))
                    nc.any.tensor_copy(qT[:, oc, :], ps[:])
                # V projection: V[k,dh] natural layout
                for kc in range(16):
                    psv = ps1.tile([P, DH], F32, tag="v", bufs=2)
                    for dc in range(8):
                        nc.tensor.matmul(psv[:], xT[:, dc, kc * P:(kc + 1) * P],
                                         wv_bf[:, dc, :],
                                         start=(dc == 0), stop=(dc == 7))
                    nc.any.tensor_copy(vaug_e[:, kc, 0:DH], psv[:])

            # ---------------- Phase C: attention (4 batches of 4 heads)
            with (
                tc.tile_pool(name="attn", bufs=1) as ap_,
                tc.tile_pool(name="pss", bufs=1, space="PSUM") as pss,
            ):
                first = True
                for bat in range(4):
                    batch_sums = []
                    for pi in range(2):
                        pr = 2 * bat + pi
                        exp_e = ap_.tile([P, 16, Q], BF16, tag="exp", bufs=5,
                                         name=f"exp_e{pr}")
                        exp_o = ap_.tile([P, 16, Q], BF16, tag="exp", bufs=5,
                                         name=f"exp_o{pr}")
                        ps_ce = pss.tile([P, 512], F32, tag="c", bufs=4,
                                         name=f"ctx_e{pr}")
                        ps_co = pss.tile([P, 512], F32, tag="c", bufs=4,
                                         name=f"ctx_o{pr}")
                        for kc2 in range(8):
                            ps_e = pss.tile([P, 2, 512], F32, tag="s", bufs=2,
                                            name="score_e")
                            ps_o = pss.tile([P, 2, 512], F32, tag="s", bufs=2,
                                            name="score_o")
                            for hf in range(2):
                                kc = 2 * kc2 + hf
                                nc.tensor.matmul(
                                    ps_e[:, hf, :],
                                    kT[0:DH, pr, kc * P:(kc + 1) * P],
                                    qT[0:DH, pr, :], start=True, stop=True)
                                nc.tensor.matmul(
                                    ps_o[:, hf, :],
                                    kT[DH:P, pr, kc * P:(kc + 1) * P],
                                    qT[DH:P, pr, :], start=True, stop=True)
                            nc.scalar.activation(
                                exp_e[:, 2 * kc2:2 * kc2 + 2, :], ps_e[:],
                                EXPF, scale=0.125)
                            nc.scalar.activation(
                                exp_o[:, 2 * kc2:2 * kc2 + 2, :], ps_o[:],
                                EXPF, scale=0.125)
                            for hf in range(2):
                                kc = 2 * kc2 + hf
                                nc.tensor.matmul(
                                    ps_ce[0:65, :], vaug_e[:, kc, :],
                                    exp_e[:, kc, :],
                                    start=(kc == 0), stop=(kc == 15))
                                nc.tensor.matmul(
                                    ps_co[0:65, :], vaug_e[:, kc, :],
                                    exp_o[:, kc, :],
                                    start=(kc == 0), stop=(kc == 15))
                        # per-pair tail: rowsums -> 1/r -> replicate -> scale
                        rs2 = ap_.tile([P, 2, 512], F32, tag="rs64", bufs=2)
                        nc.any.tensor_copy(rs2[64:65, 0, :], ps_ce[64:65, :])
                        nc.any.tensor_copy(rs2[64:65, 1, :], ps_co[64:65, :])
                        rsp = ap_.tile([2, 512], F32, tag="rs", bufs=2)
                        nc.sync.dma_start(rsp[:], rs2[64:65, :, :])
                        inv2 = ap_.tile([2, 512], F32, tag="inv", bufs=2)
                        nc.vector.reciprocal(inv2[:], rsp[:])
                        inv2b = ap_.tile([2, 512], BF16, tag="invb", bufs=2)
                        nc.any.tensor_copy(inv2b[:], inv2[:])
                        invb0 = ap_.tile([1, 2, 512], BF16, tag="invb0", bufs=2)
                        nc.sync.dma_start(invb0[:], inv2b[:])
                        for j, (ex, pc) in enumerate(
                                ((exp_e, ps_ce), (exp_o, ps_co))):
                            ps_r = pss.tile([P, 512], F32, tag="s", bufs=2,
                                            name="invrep")
                            nc.tensor.matmul(ps_r[:], allones[0:1, :],
                                             invb0[0:1, j, :],
                                             start=True, stop=True)
                            irb = ap_.tile([P, 512], BF16, tag="irb", bufs=2)
                            nc.any.tensor_copy(irb[:], ps_r[:])
                            if j == 0:
                                nc.vector.tensor_tensor(
                                    ctxT[0:DH, pr, :], pc[0:DH, :],
                                    irb[0:DH, :], op=MULT)
                            else:
                                tmpc = ap_.tile([DH, 512], BF16, tag="tmpc",
                                                bufs=2)
                                nc.vector.tensor_tensor(
                                    tmpc[:], pc[0:DH, :],
                                    irb[0:DH, :], op=MULT)
                                nc.sync.dma_start(ctxT[DH:P, pr, :], tmpc[:])
                            nc.vector.tensor_tensor(
                                ex[:], ex[:],
                                irb[:, None, :].to_broadcast([P, 16, Q]),
                                op=MULT)
                        nc.vector.tensor_tensor(exp_e[:], exp_e[:], exp_o[:],
                                                op=ADD)
                        batch_sums.append(exp_e)
                    e0, e2 = batch_sums
                    nc.vector.tensor_tensor(e0[:], e0[:], e2[:], op=ADD)
                    if first:
                        nc.any.tensor_copy(acc[:], e0[:])
                        first = False
                    else:
                        nc.vector.tensor_tensor(acc[:], acc[:], e0[:], op=ADD)

            # ---------------- Phase D: out-proj + residual + LayerNorm
            with (
                tc.tile_pool(name="fin", bufs=1) as fp,
                tc.tile_pool(name="wrk2", bufs=2) as wp2,
                tc.tile_pool(name="psf", bufs=1, space="PSUM") as psf,
            ):
                wo2 = fp.tile([P, 8, D], BF16)
                nc.sync.dma_start(wo2[:], wo_dram[:])
                g_rep = fp.tile([P, D], F32)
                b_rep = fp.tile([P, D], F32)
                nc.sync.dma_start(
                    g_rep[:],
                    g_in.ap().rearrange("(a d) -> a d", a=1).to_broadcast([P, D]))
                nc.sync.dma_start(
                    b_rep[:],
                    b_in.ap().rearrange("(a d) -> a d", a=1).to_broadcast([P, D]))
                eps_t = fp.tile([P, 1], F32)
                nc.gpsimd.memset(eps_t[:], 1e-6)
                x4 = fp.tile([P, 4, D], F32)
                for qc in range(4):
                    nc.sync.dma_start(x4[:, qc, :],
                                      x_in.ap()[qc * P:(qc + 1) * P, :])
                for qc in range(4):
                    pso = psf.tile([P, D], F32, tag="o", bufs=2)
                    for nh in range(2):
                        for dc in range(8):
                            nc.tensor.matmul(
                                pso[:, nh * 512:(nh + 1) * 512],
                                ctxT[:, dc, qc * P:(qc + 1) * P],
                                wo2[:, dc, nh * 512:(nh + 1) * 512],
                                start=(dc == 0), stop=(dc == 7))
                    y = wp2.tile([P, D], F32, tag="y")
                    ysum = wp2.tile([P, 1], F32, tag="ys")
                    nc.vector.scalar_tensor_tensor(
                        out=y[:], in0=pso[:], scalar=1.0, in1=x4[:, qc, :],
                        op0=MULT, op1=ADD, accum_out=ysum[:])
                    mu = wp2.tile([P, 1], F32, tag="mu")
                    nc.vector.tensor_scalar_mul(mu[:], ysum[:], 1.0 / D)
                    scr = wp2.tile([P, D], BF16, tag="scr")
                    vsum = wp2.tile([P, 1], F32, tag="vs")
                    nc.vector.scalar_tensor_tensor(
                        out=scr[:], in0=y[:], scalar=mu[:], in1=y[:],
                        op0=SUB, op1=MULT, accum_out=vsum[:])
                    std = wp2.tile([P, 1], F32, tag="sd")
                    nc.scalar.activation(std[:], vsum[:], SQRTF,
                                         bias=eps_t[:], scale=1.0 / D)
                    istd = wp2.tile([P, 1], F32, tag="is")
                    nc.vector.reciprocal(istd[:], std[:])
                    nrm = wp2.tile([P, D], F32, tag="nrm")
                    nc.vector.scalar_tensor_tensor(
                        out=nrm[:], in0=y[:], scalar=mu[:],
                        in1=istd[:].to_broadcast([P, D]), op0=SUB, op1=MULT)
                    nc.vector.tensor_tensor(nrm[:], nrm[:], g_rep[:], op=MULT)
                    nc.vector.tensor_tensor(nrm[:], nrm[:], b_rep[:], op=ADD)
                    nc.sync.dma_start(out_o.ap()[qc * P:(qc + 1) * P, :], nrm[:])

                # avg_weights: PE-transpose bf16 acc -> [q, k], convert +
                # scale (1/H) to fp32 on eviction
                for qb in range(4):
                    stg = fp.tile([P, S], F32, tag="stg", bufs=2)
                    for kc in range(16):
                        pst = psf.tile([P, P], BF16, tag="t", bufs=2)
                        nc.tensor.transpose(pst[:],
                                            acc[:, kc, qb * P:(qb + 1) * P],
                                            ident_b)
                        nc.scalar.mul(stg[:, kc * P:(kc + 1) * P], pst[:],
                                      1.0 / H)
                    nc.sync.dma_start(avg_o.ap()[qb * P:(qb + 1) * P, :], stg[:])

    nc.compile()
    return nc


_NC = None


def _get_nc():
    global _NC
    if _NC is None:
        _NC = build()
    return _NC


def _make_in_maps(inputs):
    x = np.ascontiguousarray(np.asarray(inputs["x"], dtype=np.float32))
    Wq = np.ascontiguousarray(np.asarray(inputs["Wq"], dtype=np.float32))
    Wk = np.ascontiguousarray(np.asarray(inputs["Wk"], dtype=np.float32))
    Wv = np.ascontiguousarray(np.asarray(inputs["Wv"], dtype=np.float32))
    Wo = np.ascontiguousarray(np.asarray(inputs["Wo"], dtype=np.float32))
    g = np.ascontiguousarray(np.asarray(inputs["ln_gamma"], dtype=np.float32))
    bt = np.ascontiguousarray(np.asarray(inputs["ln_beta"], dtype=np.float32))
    in_maps, perms = [], []
    for c in range(N_CORES):
        b_idx, j = divmod(c, 4)
        q0 = j * Q
        perm = np.concatenate([np.arange(q0, q0 + Q), np.arange(0, q0),
                               np.arange(q0 + Q, S)])
        xp = np.ascontiguousarray(x[b_idx][perm])
        in_maps.append({"x": xp, "Wq": Wq, "Wk": Wk, "Wv": Wv, "Wo": Wo,
                        "gamma": g, "beta": bt})
        perms.append(perm)
    return in_maps, perms


def run(inputs, trace=False):
    nc = _get_nc()
    in_maps, perms = _make_in_maps(inputs)
    res = run_bass_kernel_spmd(nc, in_maps, list(range(N_CORES)),
                               trace=trace, trace_cores=[0] if trace else None)
    out = np.empty((2, S, D), np.float32)
    avg = np.empty((2, S, S), np.float32)
    for c in range(N_CORES):
        b_idx, j = divmod(c, 4)
        q0 = j * Q
        out[b_idx, q0:q0 + Q] = res.results[c]["out"]
        avg[b_idx, q0:q0 + Q][:, perms[c]] = res.results[c]["avg"]
    return (out, avg), res.exec_time_ns


def kernel(**inputs):
    (out, avg), _ = run(inputs, trace=False)
    return out, avg


# revision 21
# speedup vs baseline: 1.2695x; 1.0269x over previous
import sys

if "/opt/trn_rl_repo" not in sys.path:
    sys.path.insert(0, "/opt/trn_rl_repo")

import numpy as np

import concourse.bass as bass
import concourse.mybir as mybir
import concourse.tile as tile
from concourse import bacc
from concourse.bass_utils import run_bass_kernel_spmd
from concourse.masks import make_identity

P = 128
S = 2048          # sequence length (keys) per batch
D = 1024          # d_model
H = 16            # heads
DH = 64           # d_head
Q = 512           # query rows per core
N_CORES = 8
F32 = mybir.dt.float32
BF16 = mybir.dt.bfloat16
EXPF = mybir.ActivationFunctionType.Exp
SQRTF = mybir.ActivationFunctionType.Sqrt
MULT = mybir.AluOpType.mult
ADD = mybir.AluOpType.add
SUB = mybir.AluOpType.subtract


def build():
    nc = bacc.Bacc("TRN2", target_bir_lowering=False, debug=False,
                   num_devices=N_CORES)

    x_in = nc.dram_tensor("x", [S, D], F32, kind="ExternalInput")
    wq_in = nc.dram_tensor("Wq", [D, D], F32, kind="ExternalInput")
    wk_in = nc.dram_tensor("Wk", [D, D], F32, kind="ExternalInput")
    wv_in = nc.dram_tensor("Wv", [D, DH], F32, kind="ExternalInput")
    wo_in = nc.dram_tensor("Wo", [D, D], F32, kind="ExternalInput")
    g_in = nc.dram_tensor("gamma", [D], F32, kind="ExternalInput")
    b_in = nc.dram_tensor("beta", [D], F32, kind="ExternalInput")
    out_o = nc.dram_tensor("out", [Q, D], F32, kind="ExternalOutput")
    avg_o = nc.dram_tensor("avg", [Q, S], F32, kind="ExternalOutput")

    with tile.TileContext(nc) as tc:
        with (
            tc.tile_pool(name="persist", bufs=1) as pp,
            tc.tile_pool(name="dram", bufs=1, space="DRAM") as dp,
        ):
            ident_b = pp.tile([P, P], BF16)
            allones = pp.tile([P, P], BF16)
            make_identity(nc, ident_b)
            nc.gpsimd.memset(allones[:], 1.0)

            # persistent activations
            kT = pp.tile([P, 8, S], BF16)      # K^T: [dh-chunk(2 heads), 8, S]
            qT = pp.tile([P, 8, Q], BF16)      # Q^T
            vaug_e = pp.tile([P, 16, 65], BF16)  # [V | 1]
            ctxT = pp.tile([P, 8, Q], BF16)    # normalized context^T
            acc = pp.tile([P, 16, Q], BF16)     # sum_h exp/r  (k-part, kc, q)
            wo_dram = dp.tile([P, 8, D], BF16)
            x_bf_dram = dp.tile([S, D], BF16)

            nc.gpsimd.memset(vaug_e[:], 1.0)

            # ---------------- Phase A+B: load/convert/transpose + projections
            with (
                tc.tile_pool(name="proj", bufs=1) as jp,
                tc.tile_pool(name="work", bufs=3) as wp,
                tc.tile_pool(name="ps1", bufs=1, space="PSUM") as ps1,
            ):
                xT = jp.tile([P, 8, S], BF16)   # x^T (freed after phase B)
                wq_bf = jp.tile([P, 8, D], BF16)
                wk_bf = jp.tile([P, 8, D], BF16)
                wv_bf = jp.tile([P, 8, DH], BF16)

                # weights: load + bf16 convert
                for dc in range(8):
                    for src, dst in ((wq_in, wq_bf), (wk_in, wk_bf)):
                        wf = wp.tile([P, D], F32, tag="wf")
                        nc.sync.dma_start(wf[:], src.ap()[dc * P:(dc + 1) * P, :])
                        nc.any.tensor_copy(dst[:, dc, :], wf[:])
                    wvf = wp.tile([P, DH], F32, tag="wvf")
                    nc.sync.dma_start(wvf[:], wv_in.ap()[dc * P:(dc + 1) * P, :])
                    nc.any.tensor_copy(wv_bf[:, dc, :], wvf[:])
                    # Wo -> bf16 -> DRAM scratch (reloaded in final phase)
                    wof = wp.tile([P, D], F32, tag="wf")
                    nc.sync.dma_start(wof[:], wo_in.ap()[dc * P:(dc + 1) * P, :])
                    wob = wp.tile([P, D], BF16, tag="wob")
                    nc.any.tensor_copy(wob[:], wof[:])
                    nc.sync.dma_start(wo_dram[:, dc, :], wob[:])

                # x: load rows, convert to bf16, bounce through DRAM, then
                # DMA-xbar transpose into xT (keeps TensorE free)
                for sc in range(16):
                    xf = wp.tile([P, D], F32, tag="xf")
                    nc.sync.dma_start(xf[:], x_in.ap()[sc * P:(sc + 1) * P, :])
                    xb = wp.tile([P, D], BF16, tag="xb")
                    nc.any.tensor_copy(xb[:], xf[:])
                    nc.sync.dma_start(x_bf_dram[sc * P:(sc + 1) * P, :], xb[:])
                for dc in range(8):
                    nc.sync.dma_start_transpose(xT[:, dc, :],
                                                x_bf_dram[:, dc * P:(dc + 1) * P])

                # K^T projection: kT[:,oc,s] = sum_dc Wk[dc,oc-chunk]^T x^T
                for oc in range(8):
                    for sb in range(4):
                        ps = ps1.tile([P, 512], F32, tag="kq", bufs=6)
                        for dc in range(8):
                            nc.tensor.matmul(
                                ps[:], wk_bf[:, dc, oc * P:(oc + 1) * P],
                                xT[:, dc, sb * 512:(sb + 1) * 512],
                                start=(dc == 0), stop=(dc == 7))
                        nc.any.tensor_copy(kT[:, oc, sb * 512:(sb + 1) * 512],
                                           ps[:])
                # Q^T projection (queries are rows 0:Q)
                for oc in range(8):
                    ps = ps1.tile([P, 512], F32, tag="kq", bufs=6)
                    for dc in range(8):
                        nc.tensor.matmul(ps[:], wq_bf[:, dc, oc * P:(oc + 1) * P],
                                         xT[:, dc, 0:Q],
                                         start=(dc == 0), stop=(dc == 7))
                    nc.any.tensor_copy(qT[:, oc, :], ps[:])
                # V projection: V[k,dh] natural layout
                for kc in range(16):
                    psv = ps1.tile([P, DH], F32, tag="v", bufs=2)
                    for dc in range(8):
                        nc.tensor.matmul(psv[:], xT[:, dc, kc * P:(kc + 1) * P],
                                         wv_bf[:, dc, :],
                                         start=(dc == 0), stop=(dc == 7))
                    nc.any.tensor_copy(vaug_e[:, kc, 0:DH], psv[:])

            # ---------------- Phase C: attention (4 batches of 4 heads)
            with (
                tc.tile_pool(name="attn", bufs=1) as ap_,
                tc.tile_pool(name="pss", bufs=1, space="PSUM") as pss,
            ):
                first = True
                for bat in range(4):
                    batch_sums = []
                    for pi in range(2):
                        pr = 2 * bat + pi
                        exp_e = ap_.tile([P, 16, Q], BF16, tag="exp", bufs=5,
                                         name=f"exp_e{pr}")
                        exp_o = ap_.tile([P, 16, Q], BF16, tag="exp", bufs=5,
                                         name=f"exp_o{pr}")
                        ps_ce = pss.tile([P, 512], F32, tag="c", bufs=4,
                                         name=f"ctx_e{pr}")
                        ps_co = pss.tile([P, 512], F32, tag="c", bufs=4,
                                         name=f"ctx_o{pr}")
                        for kc2 in range(8):
                            ps_e = pss.tile([P, 2, 512], F32, tag="s", bufs=2,
                                            name="score_e")
                            ps_o = pss.tile([P, 2, 512], F32, tag="s", bufs=2,
                                            name="score_o")
                            for hf in range(2):
                                kc = 2 * kc2 + hf
                                nc.tensor.matmul(
                                    ps_e[:, hf, :],
                                    kT[0:DH, pr, kc * P:(kc + 1) * P],
                                    qT[0:DH, pr, :], start=True, stop=True)
                                nc.tensor.matmul(
                                    ps_o[:, hf, :],
                                    kT[DH:P, pr, kc * P:(kc + 1) * P],
                                    qT[DH:P, pr, :], start=True, stop=True)
                            nc.scalar.activation(
                                exp_e[:, 2 * kc2:2 * kc2 + 2, :], ps_e[:],
                                EXPF, scale=0.125)
                            nc.scalar.activation(
                                exp_o[:, 2 * kc2:2 * kc2 + 2, :], ps_o[:],
                                EXPF, scale=0.125)
                            for hf in range(2):
                                kc = 2 * kc2 + hf
                                nc.tensor.matmul(
                                    ps_ce[0:65, :], vaug_e[:, kc, :],
                                    exp_e[:, kc, :],
                                    start=(kc == 0), stop=(kc == 15))
                                nc.tensor.matmul(
                                    ps_co[0:65, :], vaug_e[:, kc, :],
                                    exp_o[:, kc, :],
                                    start=(kc == 0), stop=(kc == 15))
                        # per-pair tail: rowsums -> 1/r -> replicate -> scale
                        rs2 = ap_.tile([P, 2, 512], F32, tag="rs64", bufs=2)
                        nc.vector.tensor_copy(rs2[64:65, 0, :], ps_ce[64:65, :])
                        nc.vector.tensor_copy(rs2[64:65, 1, :], ps_co[64:65, :])
                        rsp = ap_.tile([2, 512], F32, tag="rs", bufs=2)
                        nc.sync.dma_start(rsp[:], rs2[64:65, :, :])
                        inv2 = ap_.tile([2, 512], F32, tag="inv", bufs=2)
                        nc.vector.reciprocal(inv2[:], rsp[:])
                        inv2b = ap_.tile([2, 512], BF16, tag="invb", bufs=2)
                        nc.vector.tensor_copy(inv2b[:], inv2[:])
                        invb0 = ap_.tile([1, 2, 512], BF16, tag="invb0", bufs=2)
                        nc.sync.dma_start(invb0[:], inv2b[:])
                        for j, (ex, pc) in enumerate(
                                ((exp_e, ps_ce), (exp_o, ps_co))):
                            ps_r = pss.tile([P, 512], F32, tag="s", bufs=2,
                                            name="invrep")
                            nc.tensor.matmul(ps_r[:], allones[0:1, :],
                                             invb0[0:1, j, :],
                                             start=True, stop=True)
                            irb = ap_.tile([P, 512], BF16, tag="irb", bufs=2)
                            nc.vector.tensor_copy(irb[:], ps_r[:])
                            if j == 0:
                                nc.vector.tensor_tensor(
                                    ctxT[0:DH, pr, :], pc[0:DH, :],
                                    irb[0:DH, :], op=MULT)
                            else:
                                tmpc = ap_.tile([DH, 512], BF16, tag="tmpc",
                                                bufs=2)
                                nc.vector.tensor_tensor(
                                    tmpc[:], pc[0:DH, :],
                                    irb[0:DH, :], op=MULT)
                                nc.sync.dma_start(ctxT[DH:P, pr, :], tmpc[:])
                            nc.vector.tensor_tensor(
                                ex[:], ex[:],
                                irb[:, None, :].to_broadcast([P, 16, Q]),
                                op=MULT)
                        nc.vector.tensor_tensor(exp_e[:], exp_e[:], exp_o[:],
                                                op=ADD)
                        batch_sums.append(exp_e)
                    e0, e2 = batch_sums
                    nc.vector.tensor_tensor(e0[:], e0[:], e2[:], op=ADD)
                    if first:
                        nc.any.tensor_copy(acc[:], e0[:])
                        first = False
                    else:
                        nc.vector.tensor_tensor(acc[:], acc[:], e0[:], op=ADD)

            # ---------------- Phase D: out-proj + residual + LayerNorm
            with (
                tc.tile_pool(name="fin", bufs=1) as fp,
                tc.tile_pool(name="wrk2", bufs=2) as wp2,
                tc.tile_pool(name="psf", bufs=1, space="PSUM") as psf,
            ):
                wo2 = fp.tile([P, 8, D], BF16)
                nc.sync.dma_start(wo2[:], wo_dram[:])
                g_rep = fp.tile([P, D], F32)
                b_rep = fp.tile([P, D], F32)
                nc.sync.dma_start(
                    g_rep[:],
                    g_in.ap().rearrange("(a d) -> a d", a=1).to_broadcast([P, D]))
                nc.sync.dma_start(
                    b_rep[:],
                    b_in.ap().rearrange("(a d) -> a d", a=1).to_broadcast([P, D]))
                eps_t = fp.tile([P, 1], F32)
                nc.gpsimd.memset(eps_t[:], 1e-6)
                x4 = fp.tile([P, 4, D], F32)
                for qc in range(4):
                    nc.sync.dma_start(x4[:, qc, :],
                                      x_in.ap()[qc * P:(qc + 1) * P, :])
                for qc in range(4):
                    pso = psf.tile([P, D], F32, tag="o", bufs=2)
                    for nh in range(2):
                        for dc in range(8):
                            nc.tensor.matmul(
                                pso[:, nh * 512:(nh + 1) * 512],
                                ctxT[:, dc, qc * P:(qc + 1) * P],
                                wo2[:, dc, nh * 512:(nh + 1) * 512],
                                start=(dc == 0), stop=(dc == 7))
                    y = wp2.tile([P, D], F32, tag="y")
                    ysum = wp2.tile([P, 1], F32, tag="ys")
                    nc.vector.scalar_tensor_tensor(
                        out=y[:], in0=pso[:], scalar=1.0, in1=x4[:, qc, :],
                        op0=MULT, op1=ADD, accum_out=ysum[:])
                    mu = wp2.tile([P, 1], F32, tag="mu")
                    nc.vector.tensor_scalar_mul(mu[:], ysum[:], 1.0 / D)
                    scr = wp2.tile([P, D], BF16, tag="scr")
                    vsum = wp2.tile([P, 1], F32, tag="vs")
                    nc.vector.scalar_tensor_tensor(
                        out=scr[:], in0=y[:], scalar=mu[:], in1=y[:],
                        op0=SUB, op1=MULT, accum_out=vsum[:])
                    std = wp2.tile([P, 1], F32, tag="sd")
                    nc.scalar.activation(std[:], vsum[:], SQRTF,
                                         bias=eps_t[:], scale=1.0 / D)
                    istd = wp2.tile([P, 1], F32, tag="is")
                    nc.vector.reciprocal(istd[:], std[:])
                    nrm = wp2.tile([P, D], F32, tag="nrm")
                    nc.vector.scalar_tensor_tensor(
                        out=nrm[:], in0=y[:], scalar=mu[:],
                        in1=istd[:].to_broadcast([P, D]), op0=SUB, op1=MULT)
                    nc.vector.tensor_tensor(nrm[:], nrm[:], g_rep[:], op=MULT)
                    nc.vector.tensor_tensor(nrm[:], nrm[:], b_rep[:], op=ADD)
                    nc.sync.dma_start(out_o.ap()[qc * P:(qc + 1) * P, :], nrm[:])

                # avg_weights: PE-transpose bf16 acc -> [q, k], convert +
                # scale (1/H) to fp32 on eviction
                for qb in range(4):
                    stg = fp.tile([P, S], F32, tag="stg", bufs=2)
                    for kc in range(16):
                        pst = psf.tile([P, P], BF16, tag="t", bufs=2)
                        nc.tensor.transpose(pst[:],
                                            acc[:, kc, qb * P:(qb + 1) * P],
                                            ident_b)
                        nc.scalar.mul(stg[:, kc * P:(kc + 1) * P], pst[:],
                                      1.0 / H)
                    nc.sync.dma_start(avg_o.ap()[qb * P:(qb + 1) * P, :], stg[:])

    nc.compile()
    return nc


_NC = None


def _get_nc():
    global _NC
    if _NC is None:
        _NC = build()
    return _NC


def _make_in_maps(inputs):
    x = np.ascontiguousarray(np.asarray(inputs["x"], dtype=np.float32))
    Wq = np.ascontiguousarray(np.asarray(inputs["Wq"], dtype=np.float32))
    Wk = np.ascontiguousarray(np.asarray(inputs["Wk"], dtype=np.float32))
    Wv = np.ascontiguousarray(np.asarray(inputs["Wv"], dtype=np.float32))
    Wo = np.ascontiguousarray(np.asarray(inputs["Wo"], dtype=np.float32))
    g = np.ascontiguousarray(np.asarray(inputs["ln_gamma"], dtype=np.float32))
    bt = np.ascontiguousarray(np.asarray(inputs["ln_beta"], dtype=np.float32))
    in_maps, perms = [], []
    for c in range(N_CORES):
        b_idx, j = divmod(c, 4)
        q0 = j * Q
        perm = np.concatenate([np.arange(q0, q0 + Q), np.arange(0, q0),
                               np.arange(q0 + Q, S)])
        xp = np.ascontiguousarray(x[b_idx][perm])
        in_maps.append({"x": xp, "Wq": Wq, "Wk": Wk, "Wv": Wv, "Wo": Wo,
                        "gamma": g, "beta": bt})
        perms.append(perm)
    return in_maps, perms


def run(inputs, trace=False):
    nc = _get_nc()
    in_maps, perms = _make_in_maps(inputs)
    res = run_bass_kernel_spmd(nc, in_maps, list(range(N_CORES)),
                               trace=trace, trace_cores=[0] if trace else None)
    out = np.empty((2, S, D), np.float32)
    avg = np.empty((2, S, S), np.float32)
    for c in range(N_CORES):
        b_idx, j = divmod(c, 4)
        q0 = j * Q
        out[b_idx, q0:q0 + Q] = res.results[c]["out"]
        avg[b_idx, q0:q0 + Q][:, perms[c]] = res.results[c]["avg"]
    return (out, avg), res.exec_time_ns


def kernel(**inputs):
    (out, avg), _ = run(inputs, trace=False)
    return out, avg
